# revision 14
# baseline (speedup 1.0000x reference)
"""Trainium2 Bass kernel for CustomQuantizedLinear.

Computes out[b,s,o] = sum_i x[b,s,i] * ((q[o,i]-128)*0.02) + bias[o]
for x (4,2048,4096) f32, q (4096,4096) int32, bias (4096,) f32.

Sharding across 8 NeuronCores: column-parallel (8 out-feature groups,
x replicated). Each core computes a (8192 tokens, 512 out-features)
block of the flattened (8192, 4096) output.

Numerics/speed hybrid: the PE runs bf16 at 1 elem/cell/cycle, fp8
(e4m3) with perf_mode=DoubleRow at 2 virtual rows/cell/cycle. The 2e-2
rel-err budget lets the last 8 of 32 k-tiles (1024 of 4096 contraction
dims) run as 4 DoubleRow matmuls (2 k-tiles per MM), cutting the PE
stream from 32 to 28 MM-slots per token tile with measured full-scale
rel err 1.91e-2 (bf16-only is 2.4e-3).

Token tiles are processed in batches of 8: all 32 fp8 DoubleRow MMs of
the batch run first (they only need the small host-prequantized fp8
DMAs, no dequant), then the 8x24 bf16 MMs. This warms the PE during
the uint8->bf16 weight-dequant ramp at startup and leaves only 2 PE
dtype switches per batch.

Per-core dataflow:
  - bf16 w (24 k-tiles): DMA uint8 slabs -> dequant to resident bf16
    tiles, alternating ScalarE / VectorE.
  - fp8 w (8 k-tiles): host-prequantized e4m3, DMA'd directly.
  - x: one bf16 DMA per 128-token tile + one fused fp8 DMA per batch.
  - eviction: VectorE adds the DMA-broadcast bias while copying
    PSUM->SBUF, then DMA out.
"""

import numpy as np

SCALE = 0.02
ZERO_POINT = 128

B, S, K, O = 4, 2048, 4096, 4096
N_CORES = 8
TOK_GROUPS, OUT_GROUPS = 1, 8
TOK_PC = B * S // TOK_GROUPS  # 8192 tokens per core
OUT_PC = O // OUT_GROUPS      # 512 out features per core

P = 128
FREE = 512
KT = K // P          # 32 k tiles
NPAIR = 4            # fp8 DoubleRow pairs (2 k-tiles each)
KTF = 2 * NPAIR      # 8 fp8 k-tiles
KTB = KT - KTF       # 24 bf16 k-tiles
K_BF = KTB * P       # 3072
BATCH = 8            # token tiles per DR-phase/bf-phase batch

_BUILD_CACHE = {}


def _build_bass(tok_pc=TOK_PC, out_pc=OUT_PC):
    """Build + compile the per-core Bass program. Returns (nc, names)."""
    from contextlib import ExitStack

    import concourse.mybir as mybir
    import concourse.tile as tile
    from concourse import bacc

    f32 = mybir.dt.float32
    bf16 = mybir.dt.bfloat16
    u8 = mybir.dt.uint8
    f8 = mybir.dt.float8e4
    ADD = mybir.AluOpType.add
    Copy = mybir.ActivationFunctionType.Copy
    DR = mybir.MatmulPerfMode.DoubleRow

    TOKT = tok_pc // P           # 64 token tiles
    NSLAB = KTB // 2             # 12 dequant slabs of 2 k-tiles
    NB = TOKT // BATCH           # 16 batches

    nc = bacc.Bacc(None, target_bir_lowering=False)
    with tile.TileContext(nc) as tc:
        with ExitStack() as ctx:
            dram = ctx.enter_context(tc.tile_pool(name="dram", bufs=1, space="DRAM"))
            x_d = dram.tile([P, tok_pc, KTB], bf16, kind="ExternalInput", name="x_in")
            x8_d = dram.tile([P, TOKT, NPAIR, 2, P], f8, kind="ExternalInput",
                             name="x8_in")
            w_d = dram.tile([P, KTB, FREE], u8, kind="ExternalInput", name="w_in")
            w8_d = dram.tile([P, NPAIR, 2, FREE], f8, kind="ExternalInput",
                             name="w8_in")
            b_d = dram.tile([1, out_pc], f32, kind="ExternalInput", name="b_in")
            o_d = dram.tile([tok_pc, out_pc], f32, kind="ExternalOutput", name="o_out")

            const = ctx.enter_context(tc.tile_pool(name="const", bufs=1))
            stage = ctx.enter_context(tc.tile_pool(name="stage", bufs=4))
            wtp = ctx.enter_context(tc.tile_pool(name="wtp", bufs=1))
            xtp = ctx.enter_context(tc.tile_pool(name="xtp", bufs=10))
            x8p = ctx.enter_context(tc.tile_pool(name="x8p", bufs=2))
            outp = ctx.enter_context(tc.tile_pool(name="outp", bufs=4))
            psm = ctx.enter_context(tc.tile_pool(name="psm", bufs=8, space="PSUM"))

            # PE warmup: dependency-free tiny matmuls on a memset scratch
            # tile run during the input-DMA wait window, so the HAM clock
            # throttle (cold 1.2 GHz) releases before the first real MM
            warm_sb = const.tile([P, 160], bf16, name="warm_sb")
            nc.gpsimd.memset(warm_sb, 0.0)
            warm_ps = psm.tile([32, P], f32, tag="acc", name="warm_ps")
            for _ in range(45):
                nc.tensor.matmul(warm_ps, lhsT=warm_sb[:, :32],
                                 rhs=warm_sb[:, 32:160], start=True, stop=True)

            w8t = const.tile([P, NPAIR, 2, FREE], f8, name="w8t")
            wt = [wtp.tile([P, 2, FREE], bf16, name=f"wt{j}")
                  for j in range(NSLAB)]
            deq_flip = [0]

            def prep_w(j):
                """DMA + dequantize one [128, 2, 512] slab of w into wt[j]."""
                wstage = stage.tile([P, 2, FREE], u8, tag="stage", name=f"wst_{j}")
                nc.sync.dma_start(wstage, w_d[:, 2 * j:2 * j + 2, :])
                if deq_flip[0] % 2 == 0:
                    nc.scalar.activation(
                        wt[j], wstage, Copy,
                        bias=float(-ZERO_POINT * SCALE), scale=float(SCALE))
                else:
                    nc.vector.tensor_scalar(
                        wt[j], wstage, float(SCALE), float(-ZERO_POINT * SCALE),
                        mybir.AluOpType.mult, mybir.AluOpType.add)
                deq_flip[0] += 1

            def make_xt(tt):
                xt = xtp.tile([P, P, KTB], bf16, tag="xt", name=f"xt{tt}")
                nc.sync.dma_start(xt, x_d[:, tt * P:(tt + 1) * P, :])
                return xt

            def make_x8q(b, split_first=False):
                """One fused fp8-x DMA for the whole batch of tiles."""
                x8q = x8p.tile([P, BATCH, NPAIR, 2, P], f8, tag="x8q",
                               name=f"x8q{b}")
                t0 = b * BATCH
                if split_first:
                    # tile 0 alone first so MM #0 waits on only 128 KB
                    nc.sync.dma_start(x8q[:, 0, :, :, :], x8_d[:, t0, :, :, :])
                    nc.sync.dma_start(x8q[:, 1:, :, :, :],
                                      x8_d[:, t0 + 1:t0 + BATCH, :, :, :])
                else:
                    nc.sync.dma_start(x8q, x8_d[:, t0:t0 + BATCH, :, :, :])
                return x8q

            def dr_block(x8q, i, acc):
                for j in range(NPAIR):
                    nc.tensor.matmul(
                        acc, lhsT=x8q[:, i, j, :, :], rhs=w8t[:, j, :, :],
                        start=(j == 0), stop=False, perf_mode=DR)

            def bf_block(xt, acc):
                for ki in range(KTB):
                    nc.tensor.matmul(
                        acc, lhsT=xt[:, :, ki], rhs=wt[ki // 2][:, ki % 2, :],
                        start=False, stop=(ki == KTB - 1))

            def evict(tt, acc, split=False):
                ot_sb = outp.tile([P, FREE], f32, tag="outt", name=f"o_{tt}")
                if split:
                    h = P // 2
                    for r in range(2):
                        sl = slice(r * h, (r + 1) * h)
                        nc.vector.tensor_tensor(
                            ot_sb[sl, :], acc[sl, :], bias_rep[sl, :], ADD)
                        nc.sync.dma_start(
                            o_d[tt * P + r * h:tt * P + (r + 1) * h, :],
                            ot_sb[sl, :])
                else:
                    nc.vector.tensor_tensor(ot_sb, acc, bias_rep, ADD)
                    nc.sync.dma_start(o_d[tt * P:(tt + 1) * P, :], ot_sb)

            # startup DMA order: fp8 x tile 0 + fp8 w pair 0 first (MM #0
            # waits on only 256 KB), then the rest of the fp8 inputs, then
            # bf16 slabs and x tiles
            x8q0 = x8p.tile([P, BATCH, NPAIR, 2, P], f8, tag="x8q", name="x8q0")
            nc.sync.dma_start(x8q0[:, 0, :, :, :], x8_d[:, 0, :, :, :])
            for j in range(NPAIR):
                nc.sync.dma_start(w8t[:, j, :, :], w8_d[:, j, :, :])
            nc.sync.dma_start(x8q0[:, 1:3, :, :, :], x8_d[:, 1:3, :, :, :])
            nc.sync.dma_start(x8q0[:, 3:5, :, :, :], x8_d[:, 3:5, :, :, :])
            nc.sync.dma_start(x8q0[:, 5:, :, :, :], x8_d[:, 5:BATCH, :, :, :])
            prep_w(0)
            prep_w(1)
            xt_buf = {0: make_xt(0)}
            prep_w(2)
            prep_w(3)
            xt_buf[1] = make_xt(1)
            for j in range(4, 8):
                prep_w(j)
            xt_buf[2] = make_xt(2)
            for j in range(8, NSLAB):
                prep_w(j)
            xt_buf[3] = make_xt(3)
            xt_buf[4] = make_xt(4)
            bias_rep = const.tile([P, out_pc], f32, name="bias_rep")
            nc.sync.dma_start(bias_rep, b_d[0, :].partition_broadcast(P))
            for t in range(5, BATCH):
                xt_buf[t] = make_xt(t)

            x8q = x8q0
            for b in range(NB):
                tiles = list(range(b * BATCH, (b + 1) * BATCH))
                accs = {tt: psm.tile([P, FREE], f32, tag="acc", name=f"acc_{tt}")
                        for tt in tiles}
                for i, tt in enumerate(tiles):
                    dr_block(x8q, i, accs[tt])
                next_x8q = make_x8q(b + 1) if b + 1 < NB else None
                for i, tt in enumerate(tiles):
                    bf_block(xt_buf.pop(tt), accs[tt])
                    nt = (b + 1) * BATCH + i
                    if nt < TOKT:
                        xt_buf[nt] = make_xt(nt)
                    evict(tt, accs[tt], split=(tt == TOKT - 1))
                x8q = next_x8q

            names = {
                "x": x_d.tensor.name,
                "x8": x8_d.tensor.name,
                "w": w_d.tensor.name,
                "w8": w8_d.tensor.name,
                "b": b_d.tensor.name,
                "o": o_d.tensor.name,
            }

    nc.compile()
    return nc, names


def _get_built(key=(TOK_PC, OUT_PC)):
    if key not in _BUILD_CACHE:
        _BUILD_CACHE[key] = _build_bass(*key)
    return _BUILD_CACHE[key]


def make_in_maps(x, quantized_weight, bias, names,
                 tok_pc=TOK_PC, out_pc=OUT_PC, n_cores=N_CORES,
                 out_groups=OUT_GROUPS):
    import ml_dtypes

    bf16 = ml_dtypes.bfloat16
    f8 = ml_dtypes.float8_e4m3
    TOKT = tok_pc // P

    xf = np.asarray(x, dtype=np.float32).reshape(-1, K)
    # bf16 part: [tok, 3072] -> [128, tok, 24]
    xb = np.ascontiguousarray(
        xf[:, :K_BF].astype(bf16).reshape(-1, KTB, P).transpose(2, 0, 1))
    # fp8 part: [tok, 1024] -> [128, TOKT, NPAIR, 2, 128tok]
    x8 = np.ascontiguousarray(
        xf[:, K_BF:].astype(f8).reshape(TOKT, P, NPAIR, 2, P)
        .transpose(4, 0, 2, 3, 1))

    q = np.asarray(quantized_weight)
    bs = np.asarray(bias, dtype=np.float32)
    in_maps = []
    cache = {}
    for c in range(n_cores):
        og = c % out_groups
        if og not in cache:
            qog = q[og * out_pc:(og + 1) * out_pc]
            # bf16-path weights, uint8: [of, 3072] -> [128, 24, of]
            w1 = np.ascontiguousarray(
                qog[:, :K_BF].astype(np.uint8).reshape(out_pc, KTB, P)
                .transpose(2, 1, 0))
            # fp8-path weights: [of, 1024] -> [128, NPAIR, 2, of]
            wdeq = ((qog[:, K_BF:].astype(np.float32) - ZERO_POINT) * SCALE)
            w8 = np.ascontiguousarray(
                wdeq.astype(f8).reshape(out_pc, NPAIR, 2, P)
                .transpose(3, 1, 2, 0))
            cache[og] = (w1, w8, np.ascontiguousarray(
                bs[og * out_pc:(og + 1) * out_pc].reshape(1, out_pc)))
        w1, w8, bpart = cache[og]
        in_maps.append({
            names["x"]: xb,
            names["x8"]: x8,
            names["w"]: w1,
            names["w8"]: w8,
            names["b"]: bpart,
        })
    return in_maps


def assemble_out(results, names):
    out = np.empty((B * S, O), np.float32)
    for c, r in enumerate(results):
        og = c % OUT_GROUPS
        out[:, og * OUT_PC:(og + 1) * OUT_PC] = r[names["o"]]
    return out.reshape(B, S, O)


def kernel(x, quantized_weight, bias):
    from concourse.bass_utils import run_bass_kernel_spmd

    nc, names = _get_built()
    in_maps = make_in_maps(x, quantized_weight, bias, names)
    res = run_bass_kernel_spmd(nc, in_maps, core_ids=list(range(N_CORES)))
    return assemble_out(res.results, names)
